# revision 15
# baseline (speedup 1.0000x reference)
"""FFF forward pass on 8 Trainium2 NeuronCores — v9: uniform fp16 pipeline.

Data-parallel over the 16384-token batch (2048 tokens/core, 16 tiles of 128).

  - Levels 0..8 (511 nodes) dense: logits = x16@w16 + xl16@w16 + x16@wl16,
    all fp16 on PE (1 cyc/row, FWL weight loads; T1/T3 share the stationary
    x16 chunk).  x16/xl16 are host-split (x16 = fp16(x), xl16 = fp16(x-x16)),
    likewise w16/wl16, so the residual error is ~2^-22 per product —
    fp32-grade signs with no on-device split work at all.
  - Level 9: gather #1 pulls w_in[n9] (fp32); level-9 logit is a fp32 DVE dot.
  - Levels 10..11: gather #2 (keyed i10 = i9 + 512*dec9 via the bit-reversed
    relabeling) pulls w_in[n10] fp32 + 7 fp16 rows [w_out(n10), w_in(lc),
    w_in(rc), w_out(lc), w_out(rc), w_out(n9), w_out(n8)].  Level-10 logit
    fp32 on DVE; leaf logits fp16.
  - Output: dense matmul covers levels 0..7 (slots 0..255, fp16); levels
    8..11 contributions are per-token scalar*vector adds on DVE, all-fp16
    (2x mode), with a8 extracted via a one-hot masked sum.
  - 3-stage software pipeline: stage_a (logits+walk+gather1),
    stage_m (l9 + gather2), stage_b (leaf + output).
"""

import numpy as np

P = 128
D = 1024
KC = 8                 # 1024 / 128 contraction chunks
DEPTH = 11
DN = 512               # dense slots: levels 0..8 (511 nodes) + pad slot 0
N_CORES = 8
TOK = 2048             # tokens per core
NT = TOK // P          # 16 token tiles per core
BW = 7 * D             # blob2 fp16 region width (fp16 words)


def build_nc(reps=1):
    from concourse import bacc, bass, mybir, tile
    from concourse.masks import make_identity

    dt = mybir.dt
    AFT = mybir.ActivationFunctionType
    ALU = mybir.AluOpType

    nc = bacc.Bacc("TRN2", target_bir_lowering=False, debug=False)

    x_d = nc.dram_tensor("x", [TOK, D], dt.float32, kind="ExternalInput")
    x16T_d = nc.dram_tensor("x16T", [NT, P, D], dt.float16, kind="ExternalInput")
    xl16T_d = nc.dram_tensor("xl16T", [NT, P, D], dt.float16, kind="ExternalInput")
    w16_d = nc.dram_tensor("w16", [KC, P, DN], dt.float16, kind="ExternalInput")
    wl16_d = nc.dram_tensor("wl16", [KC, P, DN], dt.float16, kind="ExternalInput")
    woT_d = nc.dram_tensor("woT_dn", [2, P, D], dt.float16, kind="ExternalInput")
    blob1_d = nc.dram_tensor("blob1", [DN, D], dt.float32, kind="ExternalInput")
    blob2_d = nc.dram_tensor("blob2", [2 * DN, D + BW // 2], dt.float32,
                             kind="ExternalInput")
    out_d = nc.dram_tensor("out", [TOK, D], dt.float32, kind="ExternalOutput")

    from contextlib import ExitStack

    with tile.TileContext(nc) as tc, ExitStack() as es:
        pool_specs = [
            ("const", 1, None), ("x16", 2, None), ("xl16", 2, None),
            ("xn", 3, None),
            ("xnh", 2, None), ("nmap", 3, None), ("dec", 2, None),
            ("acts", 3, None), ("msk", 2, None), ("mskT", 2, None),
            ("gw1", 2, None), ("gw2", 2, None),
            ("dot1", 1, None), ("dot2", 1, None), ("prods", 1, None),
            ("osb", 2, None), ("tiny", 4, None), ("mid", 2, None),
            ("lps", 2, "PSUM"), ("tps", 2, "PSUM"), ("ops", 1, "PSUM"),
        ]
        pools = {}
        for pname, bufs, spc in pool_specs:
            kw = {"name": pname, "bufs": bufs}
            if spc is not None:
                kw["space"] = spc
            pools[pname] = es.enter_context(tc.tile_pool(**kw))
        (cpool, x16_pool, xl16_pool, xn_pool, xnh_pool,
         map_pool, dec_pool, acts_pool, msk_pool, mskT_pool, gw1_pool,
         gw2_pool, dot1_pool, dot2_pool, prods_pool, osb_pool, tiny_pool,
         mid_pool, lps_pool, tps_pool, ops_pool) = (
            pools[n] for n, _, _ in pool_specs)
        if True:
            identh = cpool.tile([P, P], dt.float16)
            make_identity(nc, identh[:])
            iotaf = cpool.tile([P, 256], dt.float16)
            nc.gpsimd.iota(
                iotaf[:], pattern=[[1, 256]], base=0, channel_multiplier=0,
                allow_small_or_imprecise_dtypes=True,
            )
            w16 = cpool.tile([P, KC * DN], dt.float16)
            wl16 = cpool.tile([P, KC * DN], dt.float16)
            for k in range(KC):
                sl = slice(k * DN, (k + 1) * DN)
                nc.sync.dma_start(out=w16[:, sl], in_=w16_d[k])
                nc.sync.dma_start(out=wl16[:, sl], in_=wl16_d[k])
            woT_sb = cpool.tile([P, 2 * D], dt.float16)
            nc.sync.dma_start(
                out=woT_sb[:].rearrange("p (c o) -> p c o", c=2),
                in_=woT_d[:].rearrange("c p o -> p c o"),
            )

            def stage_a(t):
                st = {}
                x16 = x16_pool.tile([P, D], dt.float16)
                nc.sync.dma_start(out=x16[:], in_=x16T_d[t])
                xl16 = xl16_pool.tile([P, D], dt.float16)
                nc.sync.dma_start(out=xl16[:], in_=xl16T_d[t])
                xn = xn_pool.tile([P, D], dt.float32)
                nc.sync.dma_start(out=xn[:], in_=x_d[t * P:(t + 1) * P, :])
                st["xn"] = xn

                # dense logits, levels 0..8 (512 slots, one PSUM bank)
                # T1 = x16@w16, T3 = x16@wl16 (shared stationary), T2 = xl16@w16
                lps = lps_pool.tile([P, DN], dt.float32, space="PSUM")
                for k in range(KC):
                    ksl = slice(k * P, (k + 1) * P)
                    for term, (lhs, rhs) in enumerate((
                        (x16, w16), (x16, wl16), (xl16, w16),
                    )):
                        nc.tensor.matmul(
                            out=lps[:],
                            lhsT=lhs[:, ksl],
                            rhs=rhs[:, k * DN:(k + 1) * DN],
                            start=(k == 0 and term == 0),
                            stop=(k == KC - 1 and term == 2),
                            skip_group_check=True,
                        )
                dec = dec_pool.tile([P, DN], dt.float16)
                nc.vector.tensor_scalar(
                    out=dec[:], in0=lps[:], scalar1=0.0, scalar2=None,
                    op0=ALU.is_gt,
                )
                acts = acts_pool.tile([P, DN], dt.float16)
                nc.scalar.activation(out=acts[:], in_=lps[:], func=AFT.Gelu)
                st["acts"] = acts

                # walk: one-hot map, level-d block at [2^d, 2^{d+1}), d<=8
                mp = map_pool.tile([P, DN], dt.float16)
                nc.vector.memset(mp[:, 0:1], 0.0)
                nc.vector.memset(mp[:, 1:2], 1.0)
                nc.vector.tensor_copy(out=mp[:, 3:4], in_=dec[:, 1:2])
                nc.vector.tensor_scalar(
                    out=mp[:, 2:3], in0=dec[:, 1:2],
                    scalar1=-1.0, scalar2=1.0, op0=ALU.mult, op1=ALU.add,
                )
                for d in range(1, 8):
                    w = 2 ** d
                    nc.vector.tensor_tensor(
                        out=mp[:, 3 * w:4 * w], in0=mp[:, w:2 * w],
                        in1=dec[:, w:2 * w], op=ALU.mult,
                    )
                    nc.vector.tensor_tensor(
                        out=mp[:, 2 * w:3 * w], in0=mp[:, w:2 * w],
                        in1=mp[:, 3 * w:4 * w], op=ALU.subtract,
                    )
                st["mp"] = mp

                # level-9 blob index: i9 = sum(mp8 * (iota + 256*dec8))
                uvec = mid_pool.tile([P, 256], dt.float16, tag="uvec")
                nc.vector.scalar_tensor_tensor(
                    out=uvec[:], in0=dec[:, 256:512], scalar=256.0,
                    in1=iotaf[:], op0=ALU.mult, op1=ALU.add,
                )
                i9f = tiny_pool.tile([P, 1], dt.float32, tag="i9f")
                nc.vector.scalar_tensor_tensor(
                    out=uvec[:], in0=uvec[:], scalar=1.0, in1=mp[:, 256:512],
                    op0=ALU.mult, op1=ALU.mult, accum_out=i9f[:],
                )
                st["i9f"] = i9f
                idx9 = tiny_pool.tile([P, 1], dt.int32, tag="idx9")
                nc.vector.tensor_copy(out=idx9[:], in_=i9f[:])

                gw1 = gw1_pool.tile([P, D], dt.float32)
                nc.gpsimd.indirect_dma_start(
                    out=gw1[:], out_offset=None, in_=blob1_d[:],
                    in_offset=bass.IndirectOffsetOnAxis(ap=idx9[:], axis=0),
                )
                st["gw1"] = gw1
                return st

            def stage_m(t, st):
                xn, gw1, i9f = st["xn"], st["gw1"], st["i9f"]
                # level-9 fp32 dot -> decision + act
                d1 = dot1_pool.tile([P, D], dt.float32, tag="dot1")
                l9 = tiny_pool.tile([P, 1], dt.float32, tag="l9")
                nc.vector.scalar_tensor_tensor(
                    out=d1[:], in0=xn[:], scalar=1.0, in1=gw1[:],
                    op0=ALU.mult, op1=ALU.mult, accum_out=l9[:],
                )
                dec9 = tiny_pool.tile([P, 1], dt.float32, tag="dec9")
                nc.vector.tensor_scalar(
                    out=dec9[:], in0=l9[:], scalar1=0.0, scalar2=None,
                    op0=ALU.is_gt,
                )
                a9 = tiny_pool.tile([P, 1], dt.float32, tag="a9")
                nc.scalar.activation(out=a9[:], in_=l9[:], func=AFT.Gelu)
                st["a9"] = a9
                # i10 = i9 + 512*dec9
                i10f = tiny_pool.tile([P, 1], dt.float32, tag="i10f")
                nc.vector.scalar_tensor_tensor(
                    out=i10f[:], in0=dec9[:], scalar=512.0, in1=i9f[:],
                    op0=ALU.mult, op1=ALU.add,
                )
                idx10 = tiny_pool.tile([P, 1], dt.int32, tag="idx10")
                nc.vector.tensor_copy(out=idx10[:], in_=i10f[:])

                gw2 = gw2_pool.tile([P, D + BW // 2], dt.float32)
                nc.gpsimd.indirect_dma_start(
                    out=gw2[:], out_offset=None, in_=blob2_d[:],
                    in_offset=bass.IndirectOffsetOnAxis(ap=idx10[:], axis=0),
                )
                st["gw2"] = gw2
                return st

            def stage_b(t, st):
                xn, acts, mp, gw2, a9 = (
                    st["xn"], st["acts"], st["mp"], st["gw2"], st["a9"])
                xnh = xnh_pool.tile([P, D], dt.float16)
                nc.scalar.copy(out=xnh[:], in_=xn[:])
                # level-10 fp32 dot -> decision + coef
                d2 = dot2_pool.tile([P, D], dt.float32, tag="dot2")
                l10 = tiny_pool.tile([P, 1], dt.float32, tag="l10")
                nc.vector.scalar_tensor_tensor(
                    out=d2[:], in0=xn[:], scalar=1.0, in1=gw2[:, 0:D],
                    op0=ALU.mult, op1=ALU.mult, accum_out=l10[:],
                )
                dec10 = tiny_pool.tile([P, 1], dt.float32, tag="dec10")
                nc.vector.tensor_scalar(
                    out=dec10[:], in0=l10[:], scalar1=0.0, scalar2=None,
                    op0=ALU.is_gt,
                )
                c10 = tiny_pool.tile([P, 1], dt.float32, tag="c10")
                nc.scalar.activation(out=c10[:], in_=l10[:], func=AFT.Gelu)

                # both leaf children's fp16 dots; zero the unchosen one
                prods = prods_pool.tile([P, 2 * D], dt.float16)
                clr = tiny_pool.tile([P, 2], dt.float32, tag="clr")
                nc.vector.scalar_tensor_tensor(
                    out=prods[:, 0:D], in0=xnh[:], scalar=1.0,
                    in1=gw2[:, D + D // 2: D + D].bitcast(dt.float16),
                    op0=ALU.mult, op1=ALU.mult, accum_out=clr[:, 0:1],
                )
                nc.vector.scalar_tensor_tensor(
                    out=prods[:, D:2 * D], in0=xnh[:], scalar=1.0,
                    in1=gw2[:, D + D: D + 3 * D // 2].bitcast(dt.float16),
                    op0=ALU.mult, op1=ALU.mult, accum_out=clr[:, 1:2],
                )
                cLR = tiny_pool.tile([P, 2], dt.float32, tag="cLR")
                nc.scalar.activation(out=cLR[:], in_=clr[:], func=AFT.Gelu)
                cl = tiny_pool.tile([P, 1], dt.float32, tag="cl")
                nc.vector.tensor_scalar(
                    out=cl[:], in0=dec10[:],
                    scalar1=-1.0, scalar2=1.0, op0=ALU.mult, op1=ALU.add,
                )
                nc.vector.tensor_tensor(
                    out=cl[:], in0=cl[:], in1=cLR[:, 0:1], op=ALU.mult
                )
                cr = tiny_pool.tile([P, 1], dt.float32, tag="cr")
                nc.vector.tensor_tensor(
                    out=cr[:], in0=dec10[:], in1=cLR[:, 1:2], op=ALU.mult
                )

                # per-token act of walked level-8 node (one-hot masked sum)
                xscr = mid_pool.tile([P, 256], dt.float16, tag="xscr")
                a8 = tiny_pool.tile([P, 1], dt.float32, tag="a8")
                nc.vector.scalar_tensor_tensor(
                    out=xscr[:], in0=acts[:, 256:512], scalar=1.0,
                    in1=mp[:, 256:512], op0=ALU.mult, op1=ALU.mult,
                    accum_out=a8[:],
                )

                # mask + transpose (fp16), dense-output slots 0..255
                msk = msk_pool.tile([P, 256], dt.float16)
                nc.gpsimd.tensor_tensor(
                    out=msk[:], in0=acts[:, 0:256], in1=mp[:, 0:256],
                    op=ALU.mult,
                )
                tps = tps_pool.tile([P, 256], dt.float16, space="PSUM")
                for c in range(2):
                    nc.tensor.transpose(
                        out=tps[:, c * P:(c + 1) * P],
                        in_=msk[:, c * P:(c + 1) * P],
                        identity=identh[:],
                    )
                mskT = mskT_pool.tile([P, 256], dt.float16)
                nc.scalar.copy(out=mskT[:], in_=tps[:])

                # dense output: levels 0..7 on PE
                ops = ops_pool.tile([P, D], dt.float32, space="PSUM")
                for c in range(2):
                    for h in range(2):
                        o0 = h * 512
                        nc.tensor.matmul(
                            out=ops[:, o0:o0 + 512],
                            lhsT=mskT[:, c * P:(c + 1) * P],
                            rhs=woT_sb[:, c * D + o0: c * D + o0 + 512],
                            start=(c == 0),
                            stop=(c == 1),
                            skip_group_check=True,
                        )
                osbh = osb_pool.tile([P, D], dt.float16, tag="osbh")
                nc.scalar.copy(out=osbh[:], in_=ops[:])
                # contributions: osb += coef * w_out[node] (per-token scalars)
                # blob2 fp16 rows: 0:wout n10, 1:win lc, 2:win rc, 3:wout lc,
                #                  4:wout rc, 5:wout n9, 6:wout n8
                for coef, boff in ((c10, 0), (cl, 3 * D), (cr, 4 * D),
                                   (a9, 5 * D)):
                    b0 = D + boff // 2
                    nc.vector.scalar_tensor_tensor(
                        out=osbh[:], in0=gw2[:, b0: b0 + D // 2].bitcast(dt.float16),
                        scalar=coef[:], in1=osbh[:], op0=ALU.mult, op1=ALU.add,
                    )
                osb = osb_pool.tile([P, D], dt.float32, tag="osb")
                b0 = D + 6 * D // 2
                nc.vector.scalar_tensor_tensor(
                    out=osb[:], in0=gw2[:, b0: b0 + D // 2].bitcast(dt.float16),
                    scalar=a8[:], in1=osbh[:], op0=ALU.mult, op1=ALU.add,
                )
                nc.sync.dma_start(out=out_d[t * P:(t + 1) * P, :], in_=osb[:])

            for _rep in range(reps):
                states = {}
                for t in range(NT + 2):
                    if t < NT:
                        states[t] = stage_a(t)
                    if 1 <= t < NT + 1:
                        states[t - 1] = stage_m(t - 1, states[t - 1])
                    if t >= 2:
                        stage_b(t - 2, states.pop(t - 2))

    nc.compile()
    return nc


def _bitrev(i, bits):
    r = 0
    for _ in range(bits):
        r = (r << 1) | (i & 1)
        i >>= 1
    return r


def _dense_perm():
    """perm[s] = heap node id stored at dense slot s (slot 0 unused)."""
    perm = np.zeros(DN, np.int64)
    for d in range(9):
        w = 2 ** d
        i = np.arange(w)
        rev = np.array([_bitrev(int(j), d) for j in i], np.int64)
        perm[w + i] = (w - 1) + rev
    return perm


def _l9_perm():
    """perm9[i] = heap id of the level-9 node stored at blob1 row i."""
    i = np.arange(512)
    rev = np.array([_bitrev(int(j), 9) for j in i], np.int64)
    return 511 + rev


def _leaf_perm():
    """lperm[i] = heap id of the level-10 node stored at blob2 row i."""
    i = np.arange(1024)
    rev = np.array([_bitrev(int(j), 10) for j in i], np.int64)
    return 1023 + rev


_DENSE_PERM = _dense_perm()
_L9_PERM = _l9_perm()
_LEAF_PERM = _leaf_perm()


def host_prep(x, w_in, w_out):
    """Build the per-core input maps (host-side transposes/splits)."""
    x = np.ascontiguousarray(x, np.float32)
    w_in = np.ascontiguousarray(w_in, np.float32)
    w_out = np.ascontiguousarray(w_out, np.float32)

    # dense (levels 0..8) weights, relabeled order, fp16 hi/lo split
    w_in_dn = np.zeros((DN, D), np.float32)
    w_in_dn[1:] = w_in[_DENSE_PERM[1:]]
    w_dnT = w_in_dn.T  # (D, DN)
    w16_f = w_dnT.astype(np.float16)
    wl16_f = (w_dnT - w16_f.astype(np.float32)).astype(np.float16)
    w16 = np.ascontiguousarray(w16_f.reshape(KC, P, DN))
    wl16 = np.ascontiguousarray(wl16_f.reshape(KC, P, DN))

    w_outT = np.ascontiguousarray(w_out.T)  # (n_nodes, D)
    woT_dn = np.zeros((DN, D), np.float32)
    woT_dn[1:] = w_outT[_DENSE_PERM[1:]]
    woT_dn = np.ascontiguousarray(
        woT_dn.reshape(4, P, D)[:2].astype(np.float16)
    )  # dense-output chunks (slots 0..255) only

    blob1 = np.ascontiguousarray(w_in[_L9_PERM])  # (512, D) fp32

    n10 = _LEAF_PERM
    lc = 2 * n10 + 1
    rc = 2 * n10 + 2
    n9 = (n10 - 1) // 2
    n8 = (n9 - 1) // 2
    blobB = np.concatenate(
        [w_outT[n10], w_in[lc], w_in[rc], w_outT[lc], w_outT[rc],
         w_outT[n9], w_outT[n8]], axis=1
    ).astype(np.float16)  # (1024, 7D) fp16
    blob2 = np.ascontiguousarray(
        np.concatenate([w_in[n10], blobB.view(np.float32)], axis=1)
    )  # (1024, D + 7D/2) fp32 words

    in_maps = []
    for c in range(N_CORES):
        xs = x[c * TOK:(c + 1) * TOK]
        xsT = xs.reshape(NT, P, KC, P).transpose(0, 3, 2, 1).reshape(NT, P, D)
        x16T = xsT.astype(np.float16)
        xl16T = (xsT - x16T.astype(np.float32)).astype(np.float16)
        in_maps.append(
            {
                "x": np.ascontiguousarray(xs),
                "x16T": np.ascontiguousarray(x16T),
                "xl16T": np.ascontiguousarray(xl16T),
                "w16": w16,
                "wl16": wl16,
                "woT_dn": woT_dn,
                "blob1": blob1,
                "blob2": blob2,
            }
        )
    return in_maps


_NC_CACHE = {}


def kernel(x, w_in, w_out, force_depth=None, **_ignored):
    from concourse.bass_utils import run_bass_kernel_spmd

    if "nc" not in _NC_CACHE:
        _NC_CACHE["nc"] = build_nc()
    nc = _NC_CACHE["nc"]

    in_maps = host_prep(np.asarray(x), np.asarray(w_in), np.asarray(w_out))
    res = run_bass_kernel_spmd(nc, in_maps, core_ids=list(range(N_CORES)))
    out = np.concatenate([res.results[c]["out"] for c in range(N_CORES)], axis=0)
    return out.astype(np.float32)


if __name__ == "__main__":
    import reference

    inputs = reference.setup_inputs()
    expected = np.asarray(reference.reference(**inputs))
    actual = kernel(**{k: np.asarray(v) for k, v in inputs.items()})
    err = np.abs(actual - expected).max()
    print("absmax err:", err)
